# revision 1
# baseline (speedup 1.0000x reference)
"""Trainium2 Bass kernel: unnormalized single-head attention block.

Computes, for x [4, 4096, 1024] and w_q/w_k/w_v/w_o [1024, 1024] (all fp32):
    q = x @ w_q ; k = x @ w_k ; v = x @ w_v
    scores = q @ k.T            (no softmax)
    out = (scores @ v) @ w_o

Sharding: 8 NeuronCores = (4 batches) x (2 sequence halves). Each core
computes the output rows for its 2048-row half of one batch. The host passes
x.T with the core's own half first ("rotated" column order); attention sums
over s are order-independent, so all per-core tensors use that rotated
order consistently.

K projection is computed for the own half only; the peer half arrives via a
masked ReduceScatter over pair groups [[0,1],[2,3],[4,5],[6,7]]: each core
stages its K into both halves of a double buffer scaled by a host-supplied
0/1 mask (own slot zeroed), so the add-reduce-scatter delivers exactly the
peer's K into a uniform buffer on every core -- no rank-dependent addressing
in the SPMD program. V is cheaper to recompute than to exchange on this
fabric (collective transfers are ~100us for 4MB and serialize on the CC
core), so each core projects V over the full rotated sequence.

Device math is bf16 with fp32 PSUM accumulation. Layout chaining (no
on-device transposes anywhere):
    qT[e,t] = wq.T @ x.T        lhsT=wq tile,   rhs=xT
    kT[e,s] = wk.T @ x.T        lhsT=wk tile,   rhs=xT
    v[s,e]  = x @ wv            lhsT=xT tile,   rhs=wv
    sT[s,t] = k @ q.T           lhsT=kT tile,   rhs=qT
    aT[e,t] = v.T @ s           lhsT=v tile,    rhs=sT
    out[t,e]= a @ wo            lhsT=aT tile,   rhs=wo
"""

import contextlib
import ctypes
import os
import sys
import types

import numpy as np

B = 4
T = 4096
D = 1024
H = T // 2          # rows per core
P = 128             # SBUF partitions
NCORES = 8
DT = D // P         # 8 tiles along any 1024 dim
ST = T // P         # 32 tiles along the full sequence
STH = H // P        # 16 own-half s-tiles
FREE = 512          # matmul moving free dim / PSUM bank width (fp32)
SBLK = T // FREE    # 8 full-sequence blocks of 512
CH = H // FREE      # 4 t-chunks per core
GROUPS = [[0, 1], [2, 3], [4, 5], [6, 7]]

_STATE = {}
LAST_RESULTS = None


def _install_axon_ntff_shim():
    """bass_utils(trace=True) under axon imports antenv.axon_hooks, which the
    agent image lacks. Provide the documented ctypes equivalent so tracing
    works; degrades to hook=None when the .so has no profile symbols."""
    try:
        import antenv.axon_hooks  # noqa: F401
        return
    except ImportError:
        pass

    so_path = "/opt/axon/libaxon_pjrt.so"

    def _make_hook():
        try:
            lib = ctypes.CDLL(so_path)
        except OSError:
            return None
        if not hasattr(lib, "axon_start_nrt_profile"):
            return None
        lib.axon_start_nrt_profile.argtypes = [
            ctypes.POINTER(ctypes.c_int64),
            ctypes.c_size_t,
        ]
        lib.axon_start_nrt_profile.restype = ctypes.c_int64
        lib.axon_stop_nrt_profile.argtypes = [ctypes.c_char_p]
        lib.axon_stop_nrt_profile.restype = ctypes.c_int64

        @contextlib.contextmanager
        def _hook(output_dir, device_ids):
            import jax

            jax.devices()
            if device_ids:
                ids = (ctypes.c_int64 * len(device_ids))(*device_ids)
                rc = lib.axon_start_nrt_profile(ids, len(device_ids))
            else:
                rc = lib.axon_start_nrt_profile(None, 0)
            if rc != 0:
                raise RuntimeError(f"axon_start_nrt_profile rc={rc}")
            try:
                yield
            finally:
                n = lib.axon_stop_nrt_profile(str(output_dir).encode())
                print(f"profile: {n} file(s) written to {output_dir}", file=sys.stderr)

        return _hook

    mod = types.ModuleType("antenv.axon_hooks")
    mod.get_axon_ntff_profile_hook = _make_hook
    mod.set_axon_ntff_profile_hook = lambda h: None
    sys.modules["antenv.axon_hooks"] = mod


def _trace_kernel(tc, xT, wq, wk, wv, wo, mask, out):
    import concourse.mybir as mybir
    from concourse.bass import ts

    nc = tc.nc
    f32 = mybir.dt.float32
    bf16 = mybir.dt.bfloat16

    with contextlib.ExitStack() as top:
        # Long-lived pools
        ktr_pool = top.enter_context(tc.tile_pool(name="ktr", bufs=DT))
        ktb_pool = top.enter_context(tc.tile_pool(name="ktb", bufs=DT))
        qt_pool = top.enter_context(tc.tile_pool(name="qt", bufs=DT))
        const_pool = top.enter_context(tc.tile_pool(name="cst", bufs=1))
        ps_pool = top.enter_context(tc.tile_pool(name="ps", bufs=8, space="PSUM"))
        dram_pool = top.enter_context(tc.tile_pool(name="cdram", bufs=4, space="DRAM"))

        # Own-half kT in row layout (filled straight from PSUM evictions);
        # peer-half kT in the same row layout, loaded from kpeer after the
        # ReduceScatter (RS output is just the peer's rows, so both sides of
        # those DMAs use large contiguous per-partition lines).
        ktr = [
            ktr_pool.tile([P, H], bf16, name=f"ktr{i}", tag="ktr") for i in range(DT)
        ]
        ktb = [
            ktb_pool.tile([P, H], bf16, name=f"ktb{i}", tag="ktb") for i in range(DT)
        ]
        qt = [qt_pool.tile([P, H], bf16, name=f"qt{i}", tag="qt") for i in range(DT)]

        mb = const_pool.tile([P, 2], f32, name="mb", tag="mb")
        nc.sync.dma_start(out=mb[:], in_=mask)

        # K-collective staging (2-core groups need Local addr space) and the
        # full-sequence V staging in local DRAM.
        kstage = dram_pool.tile([2, DT, P, H], bf16, name="kstage", tag="kst")
        kpeer = dram_pool.tile([DT, P, H], bf16, name="kpeer", tag="kp")
        vstage = dram_pool.tile([ST, P, D], bf16, name="vstage", tag="vso")

        # ---------------- setup ----------------
        with contextlib.ExitStack() as setup:
            wf_pool = setup.enter_context(tc.tile_pool(name="wf", bufs=2))
            wset_pool = setup.enter_context(tc.tile_pool(name="wset", bufs=2 * DT))
            xf_pool = setup.enter_context(tc.tile_pool(name="xf", bufs=8))
            xb_pool = setup.enter_context(tc.tile_pool(name="xb", bufs=2 * DT))
            esb_pool = setup.enter_context(tc.tile_pool(name="esb", bufs=6))

            def cast_weight(w_ap):
                tiles = []
                for i in range(DT):
                    wf = wf_pool.tile([P, D], f32, name="wf", tag="wf")
                    nc.sync.dma_start(out=wf[:], in_=w_ap[ts(i, P), :])
                    wb = wset_pool.tile([P, D], bf16, name="wb", tag="wset")
                    nc.scalar.copy(wb[:], wf[:])
                    tiles.append(wb)
                return tiles

            def load_x_block(blk):
                xb = []
                for d in range(DT):
                    xf = xf_pool.tile([P, FREE], f32, name="xf", tag="xf")
                    nc.sync.dma_start(out=xf[:], in_=xT[ts(d, P), ts(blk, FREE)])
                    xbt = xb_pool.tile([P, FREE], bf16, name="xbt", tag="xb")
                    nc.scalar.copy(xbt[:], xf[:])
                    xb.append(xbt)
                return xb


            # --- K pass (own half): fill ktr + masked staging for the RS ---
            wkb = cast_weight(wk)
            for blk in range(CH):
                xb = load_x_block(blk)
                for e in range(DT):
                    psum = ps_pool.tile([P, FREE], f32, name="psk", tag="ps")
                    for d in range(DT):
                        nc.tensor.matmul(
                            psum[:],
                            wkb[d][:, ts(e, P)],
                            xb[d][:],
                            start=(d == 0),
                            stop=(d == DT - 1),
                        )
                    nc.vector.tensor_copy(ktr[e][:, ts(blk, FREE)], psum[:])
                    for part in range(2):
                        km = esb_pool.tile([P, FREE], bf16, name="km", tag="esbk")
                        nc.vector.tensor_scalar_mul(
                            km[:], ktr[e][:, ts(blk, FREE)], mb[:, part : part + 1]
                        )
                        # Stores ride the scalar HWDGE queue so they do not
                        # back up the sync queue feeding the x loads.
                        nc.scalar.dma_start(
                            out=kstage[part, e, :, ts(blk, FREE)], in_=km[:]
                        )
            nc.gpsimd.collective_compute(
                "ReduceScatter",
                mybir.AluOpType.add,
                replica_groups=GROUPS,
                ins=[kstage.opt()],
                outs=[kpeer.opt()],
            )

            # --- combined V (full sequence) + Q (own half) pass ---
            wvb = cast_weight(wv)
            wqb = cast_weight(wq)
            for blk in range(SBLK):
                xb = load_x_block(blk)
                for ss in range(FREE // P):
                    s_tile = blk * (FREE // P) + ss
                    vt = esb_pool.tile([P, D], bf16, name="vt", tag="esbv")
                    for nh in range(2):
                        psum = ps_pool.tile([P, FREE], f32, name="psv", tag="ps")
                        for d in range(DT):
                            nc.tensor.matmul(
                                psum[:],
                                xb[d][:, ts(ss, P)],
                                wvb[d][:, ts(nh, FREE)],
                                start=(d == 0),
                                stop=(d == DT - 1),
                            )
                        nc.vector.tensor_copy(vt[:, ts(nh, FREE)], psum[:])
                    nc.scalar.dma_start(out=vstage[s_tile], in_=vt[:])
                if blk < CH:  # q projection for the own half
                    for e in range(DT):
                        psum = ps_pool.tile([P, FREE], f32, name="psq", tag="ps")
                        for d in range(DT):
                            nc.tensor.matmul(
                                psum[:],
                                wqb[d][:, ts(e, P)],
                                xb[d][:],
                                start=(d == 0),
                                stop=(d == DT - 1),
                            )
                        nc.vector.tensor_copy(qt[e][:, ts(blk, FREE)], psum[:])

        # Peer-half kT into SBUF (waits on the K ReduceScatter via tile
        # deps). Issued from the otherwise-idle SWDGE queue so the wait on
        # the collective cannot stall the sync/scalar DMA queues.
        for e in range(DT):
            nc.gpsimd.dma_start(out=ktb[e][:], in_=kpeer[e])

        # w_o cast (after setup pools release)
        wo_pool = top.enter_context(tc.tile_pool(name="wob", bufs=DT))
        wof_pool = top.enter_context(tc.tile_pool(name="wof", bufs=2))
        wob = []
        for i in range(DT):
            wf = wof_pool.tile([P, D], f32, name="wof", tag="wof")
            nc.sync.dma_start(out=wf[:], in_=wo[ts(i, P), :])
            wb = wo_pool.tile([P, D], bf16, name="wob", tag="wob")
            nc.scalar.copy(wb[:], wf[:])
            wob.append(wb)

        # ---------------- main loop over t-chunks ----------------
        sct_pool = top.enter_context(tc.tile_pool(name="sct", bufs=ST))
        att_pool = top.enter_context(tc.tile_pool(name="att", bufs=2 * DT))
        vld_pool = top.enter_context(tc.tile_pool(name="vld", bufs=6))
        ost_pool = top.enter_context(tc.tile_pool(name="ost", bufs=4))

        for c in range(CH):
            # scores^T [s, t-chunk]: own half from ktr, peer half from ktb
            sct = []
            for st in range(ST):
                psum = ps_pool.tile([P, FREE], f32, name="pss", tag="ps")
                for e in range(DT):
                    lhsT = (
                        ktr[e][:, ts(st, P)]
                        if st < STH
                        else ktb[e][:, ts(st - STH, P)]
                    )
                    nc.tensor.matmul(
                        psum[:],
                        lhsT,
                        qt[e][:, ts(c, FREE)],
                        start=(e == 0),
                        stop=(e == DT - 1),
                    )
                sc = sct_pool.tile([P, FREE], bf16, name="sc", tag="sct")
                nc.vector.tensor_copy(sc[:], psum[:])
                sct.append(sc)

            # attn^T [e, t-chunk]: all 8 PSUM banks accumulate over s, so v
            # streams through SBUF exactly once per chunk.
            att = [None] * DT
            accs = [
                ps_pool.tile([P, FREE], f32, name=f"acc{j}", tag="ps")
                for j in range(DT)
            ]
            for st in range(ST):
                vt = vld_pool.tile([P, D], bf16, name="vl", tag="vld")
                nc.sync.dma_start(out=vt[:], in_=vstage[st])
                for e in range(DT):
                    nc.tensor.matmul(
                        accs[e][:],
                        vt[:, ts(e, P)],
                        sct[st][:],
                        start=(st == 0),
                        stop=(st == ST - 1),
                    )
            for e in range(DT):
                a = att_pool.tile([P, FREE], bf16, name="at", tag="att")
                nc.vector.tensor_copy(a[:], accs[e][:])
                att[e] = a

            # output projection [t-chunk, 1024]
            for tt in range(FREE // P):
                for nh in range(2):
                    psum = ps_pool.tile([P, FREE], f32, name="pso", tag="ps")
                    for e in range(DT):
                        nc.tensor.matmul(
                            psum[:],
                            att[e][:, ts(tt, P)],
                            wob[e][:, ts(nh, FREE)],
                            start=(e == 0),
                            stop=(e == DT - 1),
                        )
                    ot = ost_pool.tile([P, FREE], f32, name="ot", tag="ost")
                    nc.scalar.copy(ot[:], psum[:])
                    row = c * FREE + tt * P
                    nc.scalar.dma_start(
                        out=out[row : row + P, ts(nh, FREE)], in_=ot[:]
                    )


def _build():
    _install_axon_ntff_shim()
    import concourse.mybir as mybir
    import concourse.tile as tile
    from concourse import bacc

    f32 = mybir.dt.float32
    nc = bacc.Bacc("TRN2", target_bir_lowering=False, debug=False, num_devices=NCORES)
    xT = nc.dram_tensor("xT", [D, T], f32, kind="ExternalInput").ap()
    wq = nc.dram_tensor("wq", [D, D], f32, kind="ExternalInput").ap()
    wk = nc.dram_tensor("wk", [D, D], f32, kind="ExternalInput").ap()
    wv = nc.dram_tensor("wv", [D, D], f32, kind="ExternalInput").ap()
    wo = nc.dram_tensor("wo", [D, D], f32, kind="ExternalInput").ap()
    mask = nc.dram_tensor("mask", [P, 2], f32, kind="ExternalInput").ap()
    out = nc.dram_tensor("out", [H, D], f32, kind="ExternalOutput").ap()

    with tile.TileContext(nc) as tc:
        _trace_kernel(tc, xT, wq, wk, wv, wo, mask, out)
    nc.compile()
    return nc


def kernel(x, w_q, w_k, w_v, w_o):
    global LAST_RESULTS
    from concourse import bass_utils

    if "nc" not in _STATE:
        _STATE["nc"] = _build()
    nc = _STATE["nc"]

    x = np.ascontiguousarray(x, dtype=np.float32)
    in_maps = []
    for core in range(NCORES):
        b, half = core // 2, core % 2
        own = x[b, half * H : (half + 1) * H]
        oth = x[b, (1 - half) * H : (2 - half) * H]
        xT = np.ascontiguousarray(np.concatenate([own, oth], axis=0).T)
        m = np.zeros((P, 2), dtype=np.float32)
        m[:, 1 - half] = 1.0  # zero own slot; pair position == half
        in_maps.append(
            {
                "xT": xT,
                "wq": np.ascontiguousarray(w_q, dtype=np.float32),
                "wk": np.ascontiguousarray(w_k, dtype=np.float32),
                "wv": np.ascontiguousarray(w_v, dtype=np.float32),
                "wo": np.ascontiguousarray(w_o, dtype=np.float32),
                "mask": m,
            }
        )

    LAST_RESULTS = bass_utils.run_bass_kernel_spmd(
        nc, in_maps, core_ids=list(range(NCORES))
    )
    out = np.empty((B, T, D), dtype=np.float32)
    for core in range(NCORES):
        b, half = core // 2, core % 2
        out[b, half * H : (half + 1) * H] = LAST_RESULTS.results[core]["out"]
    return out



# revision 2
# speedup vs baseline: 2.9318x; 2.9318x over previous
"""Trainium2 Bass kernel: unnormalized single-head attention block.

Computes, for x [4, 4096, 1024] and w_q/w_k/w_v/w_o [1024, 1024] (all fp32):
    q = x @ w_q ; k = x @ w_k ; v = x @ w_v
    scores = q @ k.T            (no softmax)
    out = (scores @ v) @ w_o

There is no softmax, so matmul associativity applies:
    out_b = x_b @ (w_q @ w_k.T @ G_b @ w_v @ w_o),   G_b = x_b.T @ x_b
which drops the arithmetic from ~413 GFLOP (two [T,T] products) to ~90 GFLOP
(everything is [D,D]-sized except the two x-sized products G and out).

Sharding: 8 NeuronCores = (4 batches) x (2 output-column halves). Each core
computes G_b over the full sequence plus AT = w_k @ w_q.T (so no on-device
transposes are needed downstream), then its 512-column slice of
C = w_v @ w_o, M1T = G @ AT (= (A @ G).T by symmetry of G), M2 = M1 @ C, and
finally out[:, half] = x_b @ M2. No collectives, no inter-core deps.

Device math is bf16 (host-cast) with fp32 PSUM accumulation. Layout chain
(lhsT's partition dim is always the contraction dim; no device transposes):
    G[d,e]   = sum_t  xn[t,d]  xn[t,e]      lhsT=xn tile,   rhs=xn tile
    AT[j,i]  = sum_d  wkT[d,j] wqT[d,i]     lhsT=wkT,       rhs=wqT
    C[e,f]   = sum_k  wvT[k,e] woh[k,f]     lhsT=wvT,       rhs=woh
    M1T[e,i] = sum_j  G[j,e]   AT[j,i]      lhsT=G,         rhs=AT
    M2[i,f]  = sum_e  M1T[e,i] C[e,f]       lhsT=M1T,       rhs=C
    out[t,f] = sum_i  xt[i,t]  M2[i,f]      lhsT=xt tile,   rhs=M2
"""

import contextlib
import ctypes
import os
import sys
import types

import numpy as np

B = 4
T = 4096
D = 1024
P = 128             # SBUF partitions
NCORES = 8
DT = D // P         # 8 tiles along any 1024 dim
ST = T // P         # 32 tiles along the sequence
FREE = 512          # PSUM bank width (fp32)
FH = D // 2         # 512 output columns per core

_STATE = {}
LAST_RESULTS = None


def _install_axon_ntff_shim():
    """bass_utils(trace=True) under axon imports antenv.axon_hooks, which the
    agent image lacks. Provide the documented ctypes equivalent so tracing
    works; degrades to hook=None when the .so has no profile symbols."""
    try:
        import antenv.axon_hooks  # noqa: F401
        return
    except ImportError:
        pass

    so_path = "/opt/axon/libaxon_pjrt.so"

    def _make_hook():
        try:
            lib = ctypes.CDLL(so_path)
        except OSError:
            return None
        if not hasattr(lib, "axon_start_nrt_profile"):
            return None
        lib.axon_start_nrt_profile.argtypes = [
            ctypes.POINTER(ctypes.c_int64),
            ctypes.c_size_t,
        ]
        lib.axon_start_nrt_profile.restype = ctypes.c_int64
        lib.axon_stop_nrt_profile.argtypes = [ctypes.c_char_p]
        lib.axon_stop_nrt_profile.restype = ctypes.c_int64

        @contextlib.contextmanager
        def _hook(output_dir, device_ids):
            import jax

            jax.devices()
            if device_ids:
                ids = (ctypes.c_int64 * len(device_ids))(*device_ids)
                rc = lib.axon_start_nrt_profile(ids, len(device_ids))
            else:
                rc = lib.axon_start_nrt_profile(None, 0)
            if rc != 0:
                raise RuntimeError(f"axon_start_nrt_profile rc={rc}")
            try:
                yield
            finally:
                n = lib.axon_stop_nrt_profile(str(output_dir).encode())
                print(f"profile: {n} file(s) written to {output_dir}", file=sys.stderr)

        return _hook

    mod = types.ModuleType("antenv.axon_hooks")
    mod.get_axon_ntff_profile_hook = _make_hook
    mod.set_axon_ntff_profile_hook = lambda h: None
    sys.modules["antenv.axon_hooks"] = mod


def _trace_kernel(tc, xn, xt, wqT, wkT, wvT, woh, out):
    import concourse.mybir as mybir
    from concourse.bass import ts

    nc = tc.nc
    f32 = mybir.dt.float32
    bf16 = mybir.dt.bfloat16

    with contextlib.ExitStack() as top:
        gsb_pool = top.enter_context(tc.tile_pool(name="gsb", bufs=DT))
        at_pool = top.enter_context(tc.tile_pool(name="at", bufs=DT))
        c_pool = top.enter_context(tc.tile_pool(name="c", bufs=DT))
        m1t_pool = top.enter_context(tc.tile_pool(name="m1t", bufs=DT))
        m2_pool = top.enter_context(tc.tile_pool(name="m2", bufs=DT))
        ps_pool = top.enter_context(tc.tile_pool(name="ps", bufs=8, space="PSUM"))

        gsb = [gsb_pool.tile([P, D], bf16, name=f"g{i}", tag="gsb") for i in range(DT)]
        at = [at_pool.tile([P, D], bf16, name=f"at{i}", tag="at") for i in range(DT)]
        csb = [c_pool.tile([P, FH], bf16, name=f"c{i}", tag="c") for i in range(DT)]
        m1t = [m1t_pool.tile([P, D], bf16, name=f"m1{i}", tag="m1t") for i in range(DT)]
        m2 = [m2_pool.tile([P, FH], bf16, name=f"m2{i}", tag="m2") for i in range(DT)]

        with contextlib.ExitStack() as setup:
            xn_pool = setup.enter_context(tc.tile_pool(name="xn", bufs=ST))
            wq_pool = setup.enter_context(tc.tile_pool(name="wq", bufs=DT))
            wk_pool = setup.enter_context(tc.tile_pool(name="wk", bufs=DT))
            wv_pool = setup.enter_context(tc.tile_pool(name="wv", bufs=DT))
            wo_pool = setup.enter_context(tc.tile_pool(name="wo", bufs=DT))

            # Weight loads: wk/wq on the sync queue ahead of xn (needed first,
            # for AT); wv/woh ride the otherwise-idle scalar queue.
            wkb = [wk_pool.tile([P, D], bf16, name=f"wk{i}", tag="wk") for i in range(DT)]
            wqb = [wq_pool.tile([P, D], bf16, name=f"wq{i}", tag="wq") for i in range(DT)]
            for i in range(DT):
                nc.sync.dma_start(out=wkb[i][:], in_=wkT[ts(i, P), :])
                nc.sync.dma_start(out=wqb[i][:], in_=wqT[ts(i, P), :])
            wvb = [wv_pool.tile([P, D], bf16, name=f"wv{i}", tag="wv") for i in range(DT)]
            wob = [wo_pool.tile([P, FH], bf16, name=f"wo{i}", tag="wo") for i in range(DT)]
            for i in range(DT):
                nc.scalar.dma_start(out=wvb[i][:], in_=wvT[ts(i, P), :])
                nc.scalar.dma_start(out=wob[i][:], in_=woh[ts(i, P), :])
            xnb = [xn_pool.tile([P, D], bf16, name=f"xn{i}", tag="xn") for i in range(ST)]
            for i in range(ST):
                nc.sync.dma_start(out=xnb[i][:], in_=xn[ts(i, P), :])

            # --- AT = wk @ wq.T (tensor engine warms up while xn streams) ---
            for half in range(2):
                pss = [
                    [ps_pool.tile([P, FREE], f32, name="psa", tag="ps") for _ in range(2)]
                    for _ in range(4)
                ]
                for dt in range(DT):
                    for j4 in range(4):
                        jb = half * 4 + j4
                        for ch in range(2):
                            nc.tensor.matmul(
                                pss[j4][ch][:],
                                wkb[dt][:, ts(jb, P)],
                                wqb[dt][:, ts(ch, FREE)],
                                start=(dt == 0),
                                stop=(dt == DT - 1),
                            )
                for j4 in range(4):
                    for ch in range(2):
                        nc.scalar.copy(at[half * 4 + j4][:, ts(ch, FREE)], pss[j4][ch][:])

            # --- C = wv @ wo[:, half] ---
            pss = [ps_pool.tile([P, FREE], f32, name="psc", tag="ps") for _ in range(DT)]
            for dt in range(DT):
                for eb in range(DT):
                    nc.tensor.matmul(
                        pss[eb][:],
                        wvb[dt][:, ts(eb, P)],
                        wob[dt][:],
                        start=(dt == 0),
                        stop=(dt == DT - 1),
                    )
            for eb in range(DT):
                nc.scalar.copy(csb[eb][:], pss[eb][:])

            # --- G = xn.T @ xn over the full sequence, 2 passes of 4 d-tiles ---
            for pp in range(2):
                pss = [
                    [ps_pool.tile([P, FREE], f32, name="psg", tag="ps") for _ in range(2)]
                    for _ in range(4)
                ]
                for tt in range(ST):
                    for d4 in range(4):
                        dt = pp * 4 + d4
                        for ch in range(2):
                            nc.tensor.matmul(
                                pss[d4][ch][:],
                                xnb[tt][:, ts(dt, P)],
                                xnb[tt][:, ts(ch, FREE)],
                                start=(tt == 0),
                                stop=(tt == ST - 1),
                            )
                for d4 in range(4):
                    for ch in range(2):
                        nc.vector.tensor_copy(
                            gsb[pp * 4 + d4][:, ts(ch, FREE)], pss[d4][ch][:]
                        )

        # --- M1T = G @ AT  (= (A@G).T since G is symmetric) ---
        for half in range(2):
            pss = [
                [ps_pool.tile([P, FREE], f32, name="psm1", tag="ps") for _ in range(2)]
                for _ in range(4)
            ]
            for jt in range(DT):
                for e4 in range(4):
                    eb = half * 4 + e4
                    for ch in range(2):
                        nc.tensor.matmul(
                            pss[e4][ch][:],
                            gsb[jt][:, ts(eb, P)],
                            at[jt][:, ts(ch, FREE)],
                            start=(jt == 0),
                            stop=(jt == DT - 1),
                        )
            for e4 in range(4):
                for ch in range(2):
                    nc.vector.tensor_copy(
                        m1t[half * 4 + e4][:, ts(ch, FREE)], pss[e4][ch][:]
                    )

        # --- M2 = M1 @ C ---
        pss = [ps_pool.tile([P, FREE], f32, name="psm2", tag="ps") for _ in range(DT)]
        for et in range(DT):
            for ib in range(DT):
                nc.tensor.matmul(
                    pss[ib][:],
                    m1t[et][:, ts(ib, P)],
                    csb[et][:],
                    start=(et == 0),
                    stop=(et == DT - 1),
                )
        for ib in range(DT):
            nc.vector.tensor_copy(m2[ib][:], pss[ib][:])

        # --- out[:, half] = x @ M2, streaming xt column-groups of 1024 ---
        xt_pool = top.enter_context(tc.tile_pool(name="xt", bufs=2 * DT))
        ost_pool = top.enter_context(tc.tile_pool(name="ost", bufs=4))
        for g in range(4):
            xtg = [
                xt_pool.tile([P, D], bf16, name=f"xt{g}_{i}", tag="xt")
                for i in range(DT)
            ]
            for it in range(DT):
                nc.gpsimd.dma_start(out=xtg[it][:], in_=xt[ts(it, P), ts(g, D)])
            pss = [ps_pool.tile([P, FREE], f32, name="pso", tag="ps") for _ in range(8)]
            for it in range(DT):
                for tb in range(8):
                    nc.tensor.matmul(
                        pss[tb][:],
                        xtg[it][:, ts(tb, P)],
                        m2[it][:],
                        start=(it == 0),
                        stop=(it == DT - 1),
                    )
            for tb in range(8):
                ot = ost_pool.tile([P, FREE], f32, name="ot", tag="ost")
                nc.scalar.copy(ot[:], pss[tb][:])
                nc.scalar.dma_start(out=out[ts(g * 8 + tb, P), :], in_=ot[:])


def _build():
    _install_axon_ntff_shim()
    import concourse.mybir as mybir
    import concourse.tile as tile
    from concourse import bacc

    f32 = mybir.dt.float32
    bf16 = mybir.dt.bfloat16
    nc = bacc.Bacc("TRN2", target_bir_lowering=False, debug=False, num_devices=NCORES)
    xn = nc.dram_tensor("xn", [T, D], bf16, kind="ExternalInput").ap()
    xt = nc.dram_tensor("xt", [D, T], bf16, kind="ExternalInput").ap()
    wqT = nc.dram_tensor("wqT", [D, D], bf16, kind="ExternalInput").ap()
    wkT = nc.dram_tensor("wkT", [D, D], bf16, kind="ExternalInput").ap()
    wvT = nc.dram_tensor("wvT", [D, D], bf16, kind="ExternalInput").ap()
    woh = nc.dram_tensor("woh", [D, FH], bf16, kind="ExternalInput").ap()
    out = nc.dram_tensor("out", [T, FH], f32, kind="ExternalOutput").ap()

    with tile.TileContext(nc) as tc:
        _trace_kernel(tc, xn, xt, wqT, wkT, wvT, woh, out)
    nc.compile()
    return nc


def kernel(x, w_q, w_k, w_v, w_o):
    global LAST_RESULTS
    import ml_dtypes
    from concourse import bass_utils

    bf16 = ml_dtypes.bfloat16

    if "nc" not in _STATE:
        _STATE["nc"] = _build()
    nc = _STATE["nc"]

    x = np.ascontiguousarray(x, dtype=np.float32)
    wqT = np.ascontiguousarray(np.asarray(w_q, dtype=np.float32).T).astype(bf16)
    wkT = np.ascontiguousarray(np.asarray(w_k, dtype=np.float32).T).astype(bf16)
    wvT = np.ascontiguousarray(np.asarray(w_v, dtype=np.float32).T).astype(bf16)
    wo = np.asarray(w_o, dtype=np.float32)
    wo_halves = [
        np.ascontiguousarray(wo[:, :FH]).astype(bf16),
        np.ascontiguousarray(wo[:, FH:]).astype(bf16),
    ]
    xn_b = [x[b].astype(bf16) for b in range(B)]
    xt_b = [np.ascontiguousarray(x[b].T).astype(bf16) for b in range(B)]

    in_maps = []
    for core in range(NCORES):
        b, fh = core // 2, core % 2
        in_maps.append(
            {
                "xn": xn_b[b],
                "xt": xt_b[b],
                "wqT": wqT,
                "wkT": wkT,
                "wvT": wvT,
                "woh": wo_halves[fh],
            }
        )

    LAST_RESULTS = bass_utils.run_bass_kernel_spmd(
        nc, in_maps, core_ids=list(range(NCORES))
    )
    out = np.empty((B, T, D), dtype=np.float32)
    for core in range(NCORES):
        b, fh = core // 2, core % 2
        out[b, :, fh * FH : (fh + 1) * FH] = LAST_RESULTS.results[core]["out"]
    return out


# revision 4
# speedup vs baseline: 3.7280x; 1.2716x over previous
"""Trainium2 Bass kernel: unnormalized single-head attention block.

Computes, for x [4, 4096, 1024] and w_q/w_k/w_v/w_o [1024, 1024] (all fp32):
    q = x @ w_q ; k = x @ w_k ; v = x @ w_v
    scores = q @ k.T            (no softmax)
    out = (scores @ v) @ w_o

There is no softmax, so matmul associativity applies:
    out_b = x_b @ (w_q @ w_k.T @ G_b @ w_v @ w_o),   G_b = x_b.T @ x_b
which drops the arithmetic from ~413 GFLOP (two [T,T] products) to ~90 GFLOP
(everything is [D,D]-sized except the two x-sized products G and out).
G is symmetric, so only its upper-triangle 128-blocks are computed directly;
the lower blocks are PE transposes of the upper ones (~3us vs ~48us of MMs).

Sharding: 8 NeuronCores = (4 batches) x (2 output-column halves). Each core
computes G_b over the full sequence plus AT = w_k @ w_q.T (so no on-device
transposes are needed downstream), then its 512-column slice of
C = w_v @ w_o, M1T = G @ AT (= (A @ G).T by symmetry of G), M2 = M1 @ C, and
finally out[:, half] = x_b @ M2. No collectives, no inter-core deps.

Device math is bf16 (host-cast) with fp32 PSUM accumulation. Layout chain
(lhsT's partition dim is always the contraction dim):
    G[d,e]   = sum_t  xn[t,d]  xn[t,e]      lhsT=xn tile,   rhs=xn tile
    AT[j,i]  = sum_d  wkT[d,j] wqT[d,i]     lhsT=wkT,       rhs=wqT
    C[e,f]   = sum_k  wvT[k,e] woh[k,f]     lhsT=wvT,       rhs=woh
    M1T[e,i] = sum_j  G[j,e]   AT[j,i]      lhsT=G,         rhs=AT
    M2[i,f]  = sum_e  M1T[e,i] C[e,f]       lhsT=M1T,       rhs=C
    out[t,f] = sum_i  xt[i,t]  M2[i,f]      lhsT=xt tile,   rhs=M2
"""

import contextlib
import ctypes
import os
import sys
import types

import numpy as np

B = 4
T = 4096
D = 1024
P = 128             # SBUF partitions
NCORES = 8
DT = D // P         # 8 tiles along any 1024 dim
ST = T // P         # 32 tiles along the sequence
FREE = 512          # PSUM bank width (fp32)
FH = D // 2         # 512 output columns per core

# Upper-triangle chunk table for symmetric G: (jt, psum chunk, e-start, width).
G_CHUNKS = []
for _jt in range(DT):
    for _c in range(2):
        _es = max(_c * FREE, _jt * P)
        _w = (_c + 1) * FREE - _es
        if _w > 0:
            G_CHUNKS.append((_jt, _c, _es, _w))
G_PASS = [[ch for ch in G_CHUNKS if ch[0] < 4], [ch for ch in G_CHUNKS if ch[0] >= 4]]

_STATE = {}
LAST_RESULTS = None


def _install_axon_ntff_shim():
    """bass_utils(trace=True) under axon imports antenv.axon_hooks, which the
    agent image lacks. Provide the documented ctypes equivalent so tracing
    works; degrades to hook=None when the .so has no profile symbols."""
    try:
        import antenv.axon_hooks  # noqa: F401
        return
    except ImportError:
        pass

    so_path = "/opt/axon/libaxon_pjrt.so"

    def _make_hook():
        try:
            lib = ctypes.CDLL(so_path)
        except OSError:
            return None
        if not hasattr(lib, "axon_start_nrt_profile"):
            return None
        lib.axon_start_nrt_profile.argtypes = [
            ctypes.POINTER(ctypes.c_int64),
            ctypes.c_size_t,
        ]
        lib.axon_start_nrt_profile.restype = ctypes.c_int64
        lib.axon_stop_nrt_profile.argtypes = [ctypes.c_char_p]
        lib.axon_stop_nrt_profile.restype = ctypes.c_int64

        @contextlib.contextmanager
        def _hook(output_dir, device_ids):
            import jax

            jax.devices()
            if device_ids:
                ids = (ctypes.c_int64 * len(device_ids))(*device_ids)
                rc = lib.axon_start_nrt_profile(ids, len(device_ids))
            else:
                rc = lib.axon_start_nrt_profile(None, 0)
            if rc != 0:
                raise RuntimeError(f"axon_start_nrt_profile rc={rc}")
            try:
                yield
            finally:
                n = lib.axon_stop_nrt_profile(str(output_dir).encode())
                print(f"profile: {n} file(s) written to {output_dir}", file=sys.stderr)

        return _hook

    mod = types.ModuleType("antenv.axon_hooks")
    mod.get_axon_ntff_profile_hook = _make_hook
    mod.set_axon_ntff_profile_hook = lambda h: None
    sys.modules["antenv.axon_hooks"] = mod


def _trace_kernel(tc, xn, xt, wqT, wkT, wvT, woh, ident, out):
    import concourse.mybir as mybir
    from concourse.bass import ts

    nc = tc.nc
    f32 = mybir.dt.float32
    bf16 = mybir.dt.bfloat16

    with contextlib.ExitStack() as top:
        gsb_pool = top.enter_context(tc.tile_pool(name="gsb", bufs=DT))
        at_pool = top.enter_context(tc.tile_pool(name="at", bufs=DT))
        c_pool = top.enter_context(tc.tile_pool(name="c", bufs=DT))
        m1t_pool = top.enter_context(tc.tile_pool(name="m1t", bufs=DT))
        m2_pool = top.enter_context(tc.tile_pool(name="m2", bufs=DT))
        id_pool = top.enter_context(tc.tile_pool(name="idp", bufs=1))
        ps_pool = top.enter_context(tc.tile_pool(name="ps", bufs=8, space="PSUM"))

        gsb = [gsb_pool.tile([P, D], bf16, name=f"g{i}", tag="gsb") for i in range(DT)]
        at = [at_pool.tile([P, D], bf16, name=f"at{i}", tag="at") for i in range(DT)]
        csb = [c_pool.tile([P, FH], bf16, name=f"c{i}", tag="c") for i in range(DT)]
        m1t = [m1t_pool.tile([P, D], bf16, name=f"m1{i}", tag="m1t") for i in range(DT)]
        m2 = [m2_pool.tile([P, FH], bf16, name=f"m2{i}", tag="m2") for i in range(DT)]
        idt = id_pool.tile([P, P], bf16, name="idt", tag="idt")
        nc.gpsimd.dma_start(out=idt[:], in_=ident)

        with contextlib.ExitStack() as setup:
            xn_pool = setup.enter_context(tc.tile_pool(name="xn", bufs=ST))
            wq_pool = setup.enter_context(tc.tile_pool(name="wq", bufs=DT))
            wk_pool = setup.enter_context(tc.tile_pool(name="wk", bufs=DT))
            wv_pool = setup.enter_context(tc.tile_pool(name="wv", bufs=DT))
            wo_pool = setup.enter_context(tc.tile_pool(name="wo", bufs=DT))

            # xn split across the sync (even tiles) and scalar (odd) queues so
            # G's streaming pass is never DMA-starved; weights follow behind.
            xnb = [xn_pool.tile([P, D], bf16, name=f"xn{i}", tag="xn") for i in range(ST)]
            for i in range(ST):
                q = nc.sync if i % 2 == 0 else nc.scalar
                q.dma_start(out=xnb[i][:], in_=xn[ts(i, P), :])
            wkb = [wk_pool.tile([P, D], bf16, name=f"wk{i}", tag="wk") for i in range(DT)]
            wqb = [wq_pool.tile([P, D], bf16, name=f"wq{i}", tag="wq") for i in range(DT)]
            for i in range(DT):
                nc.sync.dma_start(out=wkb[i][:], in_=wkT[ts(i, P), :])
                nc.sync.dma_start(out=wqb[i][:], in_=wqT[ts(i, P), :])
            wvb = [wv_pool.tile([P, D], bf16, name=f"wv{i}", tag="wv") for i in range(DT)]
            wob = [wo_pool.tile([P, FH], bf16, name=f"wo{i}", tag="wo") for i in range(DT)]
            for i in range(DT):
                nc.gpsimd.dma_start(out=wvb[i][:], in_=wvT[ts(i, P), :])
                nc.gpsimd.dma_start(out=wob[i][:], in_=woh[ts(i, P), :])

            # --- G upper triangle: two streaming passes over the sequence ---
            for chunks in G_PASS:
                pss = {
                    (jt, c): ps_pool.tile([P, FREE], f32, name="psg", tag="ps")
                    for (jt, c, es, w) in chunks
                }
                for tt in range(ST):
                    for jt, c, es, w in chunks:
                        nc.tensor.matmul(
                            pss[jt, c][:, :w],
                            xnb[tt][:, ts(jt, P)],
                            xnb[tt][:, es : es + w],
                            start=(tt == 0),
                            stop=(tt == ST - 1),
                        )
                for jt, c, es, w in chunks:
                    nc.vector.tensor_copy(gsb[jt][:, es : es + w], pss[jt, c][:, :w])

            # --- mirror the lower-triangle blocks: G[jt,eb] = G[eb,jt].T ---
            for jt in range(1, DT):
                for eb in range(jt):
                    pst = ps_pool.tile([P, P], bf16, name="pst", tag="ps")
                    nc.tensor.transpose(pst[:], gsb[eb][:, ts(jt, P)], idt[:])
                    nc.vector.tensor_copy(gsb[jt][:, ts(eb, P)], pst[:])

            # --- AT = wk @ wq.T ---
            for half in range(2):
                pss = [
                    [ps_pool.tile([P, FREE], f32, name="psa", tag="ps") for _ in range(2)]
                    for _ in range(4)
                ]
                for dt in range(DT):
                    for j4 in range(4):
                        jb = half * 4 + j4
                        for ch in range(2):
                            nc.tensor.matmul(
                                pss[j4][ch][:],
                                wkb[dt][:, ts(jb, P)],
                                wqb[dt][:, ts(ch, FREE)],
                                start=(dt == 0),
                                stop=(dt == DT - 1),
                            )
                for j4 in range(4):
                    for ch in range(2):
                        nc.scalar.copy(at[half * 4 + j4][:, ts(ch, FREE)], pss[j4][ch][:])

            # --- C = wv @ wo[:, half] ---
            pss = [ps_pool.tile([P, FREE], f32, name="psc", tag="ps") for _ in range(DT)]
            for dt in range(DT):
                for eb in range(DT):
                    nc.tensor.matmul(
                        pss[eb][:],
                        wvb[dt][:, ts(eb, P)],
                        wob[dt][:],
                        start=(dt == 0),
                        stop=(dt == DT - 1),
                    )
            for eb in range(DT):
                nc.scalar.copy(csb[eb][:], pss[eb][:])

        # --- M1T = G @ AT  (= (A@G).T since G is symmetric) ---
        for half in range(2):
            pss = [
                [ps_pool.tile([P, FREE], f32, name="psm1", tag="ps") for _ in range(2)]
                for _ in range(4)
            ]
            for jt in range(DT):
                for e4 in range(4):
                    eb = half * 4 + e4
                    for ch in range(2):
                        nc.tensor.matmul(
                            pss[e4][ch][:],
                            gsb[jt][:, ts(eb, P)],
                            at[jt][:, ts(ch, FREE)],
                            start=(jt == 0),
                            stop=(jt == DT - 1),
                        )
            for e4 in range(4):
                for ch in range(2):
                    nc.vector.tensor_copy(
                        m1t[half * 4 + e4][:, ts(ch, FREE)], pss[e4][ch][:]
                    )

        # --- M2 = M1 @ C ---
        pss = [ps_pool.tile([P, FREE], f32, name="psm2", tag="ps") for _ in range(DT)]
        for et in range(DT):
            for ib in range(DT):
                nc.tensor.matmul(
                    pss[ib][:],
                    m1t[et][:, ts(ib, P)],
                    csb[et][:],
                    start=(et == 0),
                    stop=(et == DT - 1),
                )
        for ib in range(DT):
            nc.vector.tensor_copy(m2[ib][:], pss[ib][:])

        # --- out[:, half] = x @ M2, streaming xt column-groups of 1024 ---
        xt_pool = top.enter_context(tc.tile_pool(name="xt", bufs=2 * DT))
        ost_pool = top.enter_context(tc.tile_pool(name="ost", bufs=8))
        xtg_all = []
        for g in range(4):
            xtg = [
                xt_pool.tile([P, D], bf16, name=f"xt{g}_{i}", tag="xt")
                for i in range(DT)
            ]
            for it in range(DT):
                nc.gpsimd.dma_start(out=xtg[it][:], in_=xt[ts(it, P), ts(g, D)])
            xtg_all.append(xtg)
        for g in range(4):
            xtg = xtg_all[g]
            pss = [ps_pool.tile([P, FREE], f32, name="pso", tag="ps") for _ in range(8)]
            for it in range(DT):
                for tb in range(8):
                    nc.tensor.matmul(
                        pss[tb][:],
                        xtg[it][:, ts(tb, P)],
                        m2[it][:],
                        start=(it == 0),
                        stop=(it == DT - 1),
                    )
            # Tail latency: alternate evac engines and store queues so the
            # last group's flush isn't serialized on one engine.
            for tb in range(8):
                ot = ost_pool.tile([P, FREE], f32, name="ot", tag="ost")
                if tb % 2 == 0:
                    nc.scalar.copy(ot[:], pss[tb][:])
                    nc.scalar.dma_start(out=out[ts(g * 8 + tb, P), :], in_=ot[:])
                else:
                    nc.vector.tensor_copy(ot[:], pss[tb][:])
                    nc.sync.dma_start(out=out[ts(g * 8 + tb, P), :], in_=ot[:])


def _build():
    _install_axon_ntff_shim()
    import concourse.mybir as mybir
    import concourse.tile as tile
    from concourse import bacc

    f32 = mybir.dt.float32
    bf16 = mybir.dt.bfloat16
    nc = bacc.Bacc("TRN2", target_bir_lowering=False, debug=False, num_devices=NCORES)
    xn = nc.dram_tensor("xn", [T, D], bf16, kind="ExternalInput").ap()
    xt = nc.dram_tensor("xt", [D, T], bf16, kind="ExternalInput").ap()
    wqT = nc.dram_tensor("wqT", [D, D], bf16, kind="ExternalInput").ap()
    wkT = nc.dram_tensor("wkT", [D, D], bf16, kind="ExternalInput").ap()
    wvT = nc.dram_tensor("wvT", [D, D], bf16, kind="ExternalInput").ap()
    woh = nc.dram_tensor("woh", [D, FH], bf16, kind="ExternalInput").ap()
    ident = nc.dram_tensor("ident", [P, P], bf16, kind="ExternalInput").ap()
    out = nc.dram_tensor("out", [T, FH], f32, kind="ExternalOutput").ap()

    with tile.TileContext(nc) as tc:
        _trace_kernel(tc, xn, xt, wqT, wkT, wvT, woh, ident, out)
    nc.compile()
    return nc


def kernel(x, w_q, w_k, w_v, w_o):
    global LAST_RESULTS
    import ml_dtypes
    from concourse import bass_utils

    bf16 = ml_dtypes.bfloat16

    if "nc" not in _STATE:
        _STATE["nc"] = _build()
    nc = _STATE["nc"]

    x = np.ascontiguousarray(x, dtype=np.float32)
    wqT = np.ascontiguousarray(np.asarray(w_q, dtype=np.float32).T).astype(bf16)
    wkT = np.ascontiguousarray(np.asarray(w_k, dtype=np.float32).T).astype(bf16)
    wvT = np.ascontiguousarray(np.asarray(w_v, dtype=np.float32).T).astype(bf16)
    wo = np.asarray(w_o, dtype=np.float32)
    wo_halves = [
        np.ascontiguousarray(wo[:, :FH]).astype(bf16),
        np.ascontiguousarray(wo[:, FH:]).astype(bf16),
    ]
    ident = np.eye(P, dtype=np.float32).astype(bf16)
    xn_b = [x[b].astype(bf16) for b in range(B)]
    xt_b = [np.ascontiguousarray(x[b].T).astype(bf16) for b in range(B)]

    in_maps = []
    for core in range(NCORES):
        b, fh = core // 2, core % 2
        in_maps.append(
            {
                "xn": xn_b[b],
                "xt": xt_b[b],
                "wqT": wqT,
                "wkT": wkT,
                "wvT": wvT,
                "woh": wo_halves[fh],
                "ident": ident,
            }
        )

    LAST_RESULTS = bass_utils.run_bass_kernel_spmd(
        nc, in_maps, core_ids=list(range(NCORES))
    )
    out = np.empty((B, T, D), dtype=np.float32)
    for core in range(NCORES):
        b, fh = core // 2, core % 2
        out[b, :, fh * FH : (fh + 1) * FH] = LAST_RESULTS.results[core]["out"]
    return out


# revision 14
# speedup vs baseline: 3.8450x; 1.0314x over previous
"""Trainium2 Bass kernel: unnormalized single-head attention block.

Computes, for x [4, 4096, 1024] and w_q/w_k/w_v/w_o [1024, 1024] (all fp32):
    q = x @ w_q ; k = x @ w_k ; v = x @ w_v
    scores = q @ k.T            (no softmax)
    out = (scores @ v) @ w_o

There is no softmax, so matmul associativity applies:
    out_b = x_b @ (w_q @ w_k.T @ G_b @ w_v @ w_o),   G_b = x_b.T @ x_b
which drops the arithmetic from ~413 GFLOP (two [T,T] products) to ~90 GFLOP.
G is symmetric, so only its upper-triangle 128-blocks are computed directly;
the lower blocks are PE transposes of the upper ones (~3us vs ~48us of MMs).
The weight chain is right-associated against the core's 512-column slice of
w_o, so every factor is a [D,D] x [D,512] product (4 x 13.7us; no [D,D]x[D,D]
products at all):
    M2 = wq @ (wk.T @ (G @ (wv @ woh)))

Sharding: 8 NeuronCores = (4 batches) x (2 output-column halves). Each core
computes G_b over the full sequence, its M2 slice, and out[:, half] = x @ M2.
No collectives, no inter-core deps. (A pair-AllReduce version that halves the
G work was measured SLOWER: the CC op costs ~29us at ~40GB/s bus plus ~25us
of startup latency, which cannot hide behind the ~41us of independent work.)

Device math is bf16 (host-cast) with fp32 PSUM accumulation. Layout chain
(lhsT's partition dim is always the contraction dim):
    G[d,e]   = sum_t  xn[t,d]  xn[t,e]      lhsT=xn tile,   rhs=xn tile
    N1[e,f]  = sum_k  wvT[k,e] woh[k,f]     lhsT=wvT,       rhs=woh
    N2[d,f]  = sum_e  G[d,e]   N1[e,f]      lhsT=G (sym),   rhs=N1
    K1[c,f]  = sum_d  wk[d,c]  N2[d,f]      lhsT=wk,        rhs=N2
    M2[i,f]  = sum_c  wqT[c,i] K1[c,f]      lhsT=wqT,       rhs=K1
    out[t,f] = sum_i  xt[i,t]  M2[i,f]      lhsT=xt tile,   rhs=M2
"""

import contextlib
import ctypes
import os
import sys
import types

import numpy as np

B = 4
T = 4096
D = 1024
P = 128             # SBUF partitions
NCORES = 8
DT = D // P         # 8 tiles along any 1024 dim
ST = T // P         # 32 tiles along the sequence
FREE = 512          # PSUM bank width (fp32)
FH = D // 2         # 512 output columns per core

# Upper-triangle chunk table for symmetric G: (jt, psum chunk, e-start, width).
G_CHUNKS = []
for _jt in range(DT):
    for _c in range(2):
        _es = max(_c * FREE, _jt * P)
        _w = (_c + 1) * FREE - _es
        if _w > 0:
            G_CHUNKS.append((_jt, _c, _es, _w))
G_PASS = [[ch for ch in G_CHUNKS if ch[0] < 4], [ch for ch in G_CHUNKS if ch[0] >= 4]]

_STATE = {}
LAST_RESULTS = None


def _install_axon_ntff_shim():
    """bass_utils(trace=True) under axon imports antenv.axon_hooks, which the
    agent image lacks. Provide the documented ctypes equivalent so tracing
    works; degrades to hook=None when the .so has no profile symbols."""
    try:
        import antenv.axon_hooks  # noqa: F401
        return
    except ImportError:
        pass

    so_path = "/opt/axon/libaxon_pjrt.so"

    def _make_hook():
        try:
            lib = ctypes.CDLL(so_path)
        except OSError:
            return None
        if not hasattr(lib, "axon_start_nrt_profile"):
            return None
        lib.axon_start_nrt_profile.argtypes = [
            ctypes.POINTER(ctypes.c_int64),
            ctypes.c_size_t,
        ]
        lib.axon_start_nrt_profile.restype = ctypes.c_int64
        lib.axon_stop_nrt_profile.argtypes = [ctypes.c_char_p]
        lib.axon_stop_nrt_profile.restype = ctypes.c_int64

        @contextlib.contextmanager
        def _hook(output_dir, device_ids):
            import jax

            jax.devices()
            if device_ids:
                ids = (ctypes.c_int64 * len(device_ids))(*device_ids)
                rc = lib.axon_start_nrt_profile(ids, len(device_ids))
            else:
                rc = lib.axon_start_nrt_profile(None, 0)
            if rc != 0:
                raise RuntimeError(f"axon_start_nrt_profile rc={rc}")
            try:
                yield
            finally:
                n = lib.axon_stop_nrt_profile(str(output_dir).encode())
                print(f"profile: {n} file(s) written to {output_dir}", file=sys.stderr)

        return _hook

    mod = types.ModuleType("antenv.axon_hooks")
    mod.get_axon_ntff_profile_hook = _make_hook
    mod.set_axon_ntff_profile_hook = lambda h: None
    sys.modules["antenv.axon_hooks"] = mod


def _trace_kernel(tc, xn, xt, wqT, wk, wvT, woh, ident, out):
    import concourse.mybir as mybir
    from concourse.bass import ts

    nc = tc.nc
    f32 = mybir.dt.float32
    bf16 = mybir.dt.bfloat16

    with contextlib.ExitStack() as top:
        gsb_pool = top.enter_context(tc.tile_pool(name="gsb", bufs=DT))
        n1_pool = top.enter_context(tc.tile_pool(name="n1", bufs=DT))
        n2_pool = top.enter_context(tc.tile_pool(name="n2", bufs=DT))
        k1_pool = top.enter_context(tc.tile_pool(name="k1", bufs=DT))
        m2_pool = top.enter_context(tc.tile_pool(name="m2", bufs=DT))
        id_pool = top.enter_context(tc.tile_pool(name="idp", bufs=1))
        wq_pool = top.enter_context(tc.tile_pool(name="wq", bufs=DT))
        wk_pool = top.enter_context(tc.tile_pool(name="wk", bufs=DT))
        wv_pool = top.enter_context(tc.tile_pool(name="wv", bufs=DT))
        wo_pool = top.enter_context(tc.tile_pool(name="wo", bufs=DT))
        xt_pool = top.enter_context(tc.tile_pool(name="xt", bufs=12))
        ost_pool = top.enter_context(tc.tile_pool(name="ost", bufs=4))
        ps_pool = top.enter_context(tc.tile_pool(name="ps", bufs=8, space="PSUM"))

        gsb = [gsb_pool.tile([P, D], bf16, name=f"g{i}", tag="gsb") for i in range(DT)]
        n1 = [n1_pool.tile([P, FH], bf16, name=f"n1_{i}", tag="n1") for i in range(DT)]
        n2 = [n2_pool.tile([P, FH], bf16, name=f"n2_{i}", tag="n2") for i in range(DT)]
        k1 = [k1_pool.tile([P, FH], bf16, name=f"k1_{i}", tag="k1") for i in range(DT)]
        m2 = [m2_pool.tile([P, FH], bf16, name=f"m2_{i}", tag="m2") for i in range(DT)]
        idt = id_pool.tile([P, P], bf16, name="idt", tag="idt")

        # DMA queues: gpsimd carries the N1 inputs first (the tensor engine's
        # opening act), then the xt stream for the output stage; xn splits
        # across sync/scalar so G is never DMA-starved; wk/wqT follow behind.
        nc.gpsimd.dma_start(out=idt[:], in_=ident)
        wvb = [wv_pool.tile([P, D], bf16, name=f"wv{i}", tag="wv") for i in range(DT)]
        wob = [wo_pool.tile([P, FH], bf16, name=f"wo{i}", tag="wo") for i in range(DT)]
        for i in range(DT):
            nc.gpsimd.dma_start(out=wvb[i][:], in_=wvT[ts(i, P), :])
            nc.gpsimd.dma_start(out=wob[i][:], in_=woh[ts(i, P), :])

        with contextlib.ExitStack() as setup:
            xn_pool = setup.enter_context(tc.tile_pool(name="xn", bufs=ST))
            xnb = [xn_pool.tile([P, D], bf16, name=f"xn{i}", tag="xn") for i in range(ST)]
            for i in range(ST):
                q = nc.sync if i % 2 == 0 else nc.scalar
                q.dma_start(out=xnb[i][:], in_=xn[ts(i, P), :])
            wkb = [wk_pool.tile([P, D], bf16, name=f"wk{i}", tag="wk") for i in range(DT)]
            wqb = [wq_pool.tile([P, D], bf16, name=f"wq{i}", tag="wq") for i in range(DT)]
            for i in range(DT):
                nc.sync.dma_start(out=wkb[i][:], in_=wk[ts(i, P), :])
                nc.scalar.dma_start(out=wqb[i][:], in_=wqT[ts(i, P), :])
            xtg_all = []
            for g in range(4):
                xtg = [
                    xt_pool.tile([P, D], bf16, name=f"xt{g}_{i}", tag="xt")
                    for i in range(DT)
                ]
                for it in range(DT):
                    nc.gpsimd.dma_start(out=xtg[it][:], in_=xt[ts(it, P), ts(g, D)])
                xtg_all.append(xtg)

            # --- N1 = wv @ wo[:, half] (tensor warms up while xn streams) ---
            pss = [ps_pool.tile([P, FREE], f32, name="psn1", tag="ps") for _ in range(DT)]
            for dt in range(DT):
                for eb in range(DT):
                    nc.tensor.matmul(
                        pss[eb][:],
                        wvb[dt][:, ts(eb, P)],
                        wob[dt][:],
                        start=(dt == 0),
                        stop=(dt == DT - 1),
                    )
            for eb in range(DT):
                nc.vector.tensor_copy(n1[eb][:], pss[eb][:])

            # --- G upper triangle: two streaming passes over the sequence ---
            for chunks in G_PASS:
                pss = {
                    (jt, c): ps_pool.tile([P, FREE], f32, name="psg", tag="ps")
                    for (jt, c, es, w) in chunks
                }
                for tt in range(ST):
                    for jt, c, es, w in chunks:
                        nc.tensor.matmul(
                            pss[jt, c][:, :w],
                            xnb[tt][:, ts(jt, P)],
                            xnb[tt][:, es : es + w],
                            start=(tt == 0),
                            stop=(tt == ST - 1),
                        )
                for jt, c, es, w in chunks:
                    nc.vector.tensor_copy(gsb[jt][:, es : es + w], pss[jt, c][:, :w])

            # --- mirror the lower-triangle blocks: G[jt,eb] = G[eb,jt].T ---
            for jt in range(1, DT):
                for eb in range(jt):
                    pst = ps_pool.tile([P, P], bf16, name="pst", tag="ps")
                    nc.tensor.transpose(pst[:], gsb[eb][:, ts(jt, P)], idt[:])
                    nc.vector.tensor_copy(gsb[jt][:, ts(eb, P)], pst[:])

        # --- N2 = G @ N1 (lhsT=G works because G is symmetric) ---
        pss = [ps_pool.tile([P, FREE], f32, name="psn2", tag="ps") for _ in range(DT)]
        for et in range(DT):
            for db in range(DT):
                nc.tensor.matmul(
                    pss[db][:],
                    gsb[et][:, ts(db, P)],
                    n1[et][:],
                    start=(et == 0),
                    stop=(et == DT - 1),
                )
        for db in range(DT):
            nc.scalar.copy(n2[db][:], pss[db][:])

        # --- K1 = wk.T @ N2 ---
        pss = [ps_pool.tile([P, FREE], f32, name="psk1", tag="ps") for _ in range(DT)]
        for dt in range(DT):
            for cb in range(DT):
                nc.tensor.matmul(
                    pss[cb][:],
                    wkb[dt][:, ts(cb, P)],
                    n2[dt][:],
                    start=(dt == 0),
                    stop=(dt == DT - 1),
                )
        for cb in range(DT):
            nc.vector.tensor_copy(k1[cb][:], pss[cb][:])

        # --- M2 = wq @ K1 ---
        pss = [ps_pool.tile([P, FREE], f32, name="psm2", tag="ps") for _ in range(DT)]
        for ct in range(DT):
            for ib in range(DT):
                nc.tensor.matmul(
                    pss[ib][:],
                    wqb[ct][:, ts(ib, P)],
                    k1[ct][:],
                    start=(ct == 0),
                    stop=(ct == DT - 1),
                )
        for ib in range(DT):
            nc.scalar.copy(m2[ib][:], pss[ib][:])

        # --- out[:, half] = x @ M2, streaming xt column-groups of 1024 ---
        for g in range(4):
            xtg = xtg_all[g]
            pss = [ps_pool.tile([P, FREE], f32, name="pso", tag="ps") for _ in range(8)]
            for it in range(DT):
                for tb in range(8):
                    nc.tensor.matmul(
                        pss[tb][:],
                        xtg[it][:, ts(tb, P)],
                        m2[it][:],
                        start=(it == 0),
                        stop=(it == DT - 1),
                    )
            # Tail latency: alternate evac engines and store queues so the
            # last group's flush isn't serialized on one engine.
            for tb in range(8):
                ot = ost_pool.tile([P, FREE], f32, name="ot", tag="ost")
                if tb % 2 == 0:
                    nc.scalar.copy(ot[:], pss[tb][:])
                    nc.scalar.dma_start(out=out[ts(g * 8 + tb, P), :], in_=ot[:])
                else:
                    nc.vector.tensor_copy(ot[:], pss[tb][:])
                    nc.sync.dma_start(out=out[ts(g * 8 + tb, P), :], in_=ot[:])


def _build():
    _install_axon_ntff_shim()
    import concourse.mybir as mybir
    import concourse.tile as tile
    from concourse import bacc

    f32 = mybir.dt.float32
    bf16 = mybir.dt.bfloat16
    nc = bacc.Bacc("TRN2", target_bir_lowering=False, debug=False, num_devices=NCORES)
    xn = nc.dram_tensor("xn", [T, D], bf16, kind="ExternalInput").ap()
    xt = nc.dram_tensor("xt", [D, T], bf16, kind="ExternalInput").ap()
    wqT = nc.dram_tensor("wqT", [D, D], bf16, kind="ExternalInput").ap()
    wk = nc.dram_tensor("wk", [D, D], bf16, kind="ExternalInput").ap()
    wvT = nc.dram_tensor("wvT", [D, D], bf16, kind="ExternalInput").ap()
    woh = nc.dram_tensor("woh", [D, FH], bf16, kind="ExternalInput").ap()
    ident = nc.dram_tensor("ident", [P, P], bf16, kind="ExternalInput").ap()
    out = nc.dram_tensor("out", [T, FH], f32, kind="ExternalOutput").ap()

    with tile.TileContext(nc) as tc:
        _trace_kernel(tc, xn, xt, wqT, wk, wvT, woh, ident, out)
    nc.compile()
    return nc


def kernel(x, w_q, w_k, w_v, w_o):
    global LAST_RESULTS
    import ml_dtypes
    from concourse import bass_utils

    bf16 = ml_dtypes.bfloat16

    if "nc" not in _STATE:
        _STATE["nc"] = _build()
    nc = _STATE["nc"]

    x = np.ascontiguousarray(x, dtype=np.float32)
    wqT = np.ascontiguousarray(np.asarray(w_q, dtype=np.float32).T).astype(bf16)
    wkb = np.ascontiguousarray(np.asarray(w_k, dtype=np.float32)).astype(bf16)
    wvT = np.ascontiguousarray(np.asarray(w_v, dtype=np.float32).T).astype(bf16)
    wo = np.asarray(w_o, dtype=np.float32)
    wo_halves = [
        np.ascontiguousarray(wo[:, :FH]).astype(bf16),
        np.ascontiguousarray(wo[:, FH:]).astype(bf16),
    ]
    ident = np.eye(P, dtype=np.float32).astype(bf16)
    xn_b = [x[b].astype(bf16) for b in range(B)]
    xt_b = [np.ascontiguousarray(x[b].T).astype(bf16) for b in range(B)]

    in_maps = []
    for core in range(NCORES):
        b, fh = core // 2, core % 2
        in_maps.append(
            {
                "xn": xn_b[b],
                "xt": xt_b[b],
                "wqT": wqT,
                "wk": wkb,
                "wvT": wvT,
                "woh": wo_halves[fh],
                "ident": ident,
            }
        )

    LAST_RESULTS = bass_utils.run_bass_kernel_spmd(
        nc, in_maps, core_ids=list(range(NCORES))
    )
    out = np.empty((B, T, D), dtype=np.float32)
    for core in range(NCORES):
        b, fh = core // 2, core % 2
        out[b, :, fh * FH : (fh + 1) * FH] = LAST_RESULTS.results[core]["out"]
    return out


# revision 17
# speedup vs baseline: 4.0711x; 1.0588x over previous
"""Trainium2 Bass kernel: unnormalized single-head attention block.

Computes, for x [4, 4096, 1024] and w_q/w_k/w_v/w_o [1024, 1024] (all fp32):
    q = x @ w_q ; k = x @ w_k ; v = x @ w_v
    scores = q @ k.T            (no softmax)
    out = (scores @ v) @ w_o

There is no softmax, so matmul associativity applies:
    out_b = x_b @ (w_q @ w_k.T @ G_b @ w_v @ w_o),   G_b = x_b.T @ x_b
which drops the arithmetic from ~413 GFLOP (two [T,T] products) to ~90 GFLOP.
G is symmetric, so only its upper-triangle 128-blocks are computed directly;
the lower blocks are PE transposes of the upper ones (~3us vs ~48us of MMs).
The weight chain is right-associated against the core's 512-column slice of
w_o, so every factor is a [D,D] x [D,512] product (4 x 13.7us; no [D,D]x[D,D]
products at all):
    M2 = wq @ (wk.T @ (G @ (wv @ woh)))

Sharding: 8 NeuronCores = (4 batches) x (2 output-column halves). Each core
computes G_b over the full sequence, its M2 slice, and out[:, half] = x @ M2.
No collectives, no inter-core deps. (A pair-AllReduce version that halves the
G work was measured SLOWER: the CC op costs ~29us at ~40GB/s bus plus ~25us
of startup latency, which cannot hide behind the ~41us of independent work.)

Device math is bf16 (host-cast) with fp32 PSUM accumulation. Layout chain
(lhsT's partition dim is always the contraction dim):
    G[d,e]   = sum_t  xn[t,d]  xn[t,e]      lhsT=xn tile,   rhs=xn tile
    N1[e,f]  = sum_k  wvT[k,e] woh[k,f]     lhsT=wvT,       rhs=woh
    N2[d,f]  = sum_e  G[d,e]   N1[e,f]      lhsT=G (sym),   rhs=N1
    K1[c,f]  = sum_d  wk[d,c]  N2[d,f]      lhsT=wk,        rhs=N2
    M2[i,f]  = sum_c  wqT[c,i] K1[c,f]      lhsT=wqT,       rhs=K1
    out[t,f] = sum_i  xt[i,t]  M2[i,f]      lhsT=xt tile,   rhs=M2
"""

import contextlib
import ctypes
import os
import sys
import types

import numpy as np

B = 4
T = 4096
D = 1024
P = 128             # SBUF partitions
NCORES = 8
DT = D // P         # 8 tiles along any 1024 dim
ST = T // P         # 32 tiles along the sequence
FREE = 512          # PSUM bank width (fp32)
FH = D // 2         # 512 output columns per core

# Upper-triangle chunk table for symmetric G: (jt, psum chunk, e-start, width).
G_CHUNKS = []
for _jt in range(DT):
    for _c in range(2):
        _es = max(_c * FREE, _jt * P)
        _w = (_c + 1) * FREE - _es
        if _w > 0:
            G_CHUNKS.append((_jt, _c, _es, _w))
G_PASS = [[ch for ch in G_CHUNKS if ch[0] < 4], [ch for ch in G_CHUNKS if ch[0] >= 4]]

_STATE = {}
LAST_RESULTS = None


def _install_axon_ntff_shim():
    """bass_utils(trace=True) under axon imports antenv.axon_hooks, which the
    agent image lacks. Provide the documented ctypes equivalent so tracing
    works; degrades to hook=None when the .so has no profile symbols."""
    try:
        import antenv.axon_hooks  # noqa: F401
        return
    except ImportError:
        pass

    so_path = "/opt/axon/libaxon_pjrt.so"

    def _make_hook():
        try:
            lib = ctypes.CDLL(so_path)
        except OSError:
            return None
        if not hasattr(lib, "axon_start_nrt_profile"):
            return None
        lib.axon_start_nrt_profile.argtypes = [
            ctypes.POINTER(ctypes.c_int64),
            ctypes.c_size_t,
        ]
        lib.axon_start_nrt_profile.restype = ctypes.c_int64
        lib.axon_stop_nrt_profile.argtypes = [ctypes.c_char_p]
        lib.axon_stop_nrt_profile.restype = ctypes.c_int64

        @contextlib.contextmanager
        def _hook(output_dir, device_ids):
            import jax

            jax.devices()
            if device_ids:
                ids = (ctypes.c_int64 * len(device_ids))(*device_ids)
                rc = lib.axon_start_nrt_profile(ids, len(device_ids))
            else:
                rc = lib.axon_start_nrt_profile(None, 0)
            if rc != 0:
                raise RuntimeError(f"axon_start_nrt_profile rc={rc}")
            try:
                yield
            finally:
                n = lib.axon_stop_nrt_profile(str(output_dir).encode())
                print(f"profile: {n} file(s) written to {output_dir}", file=sys.stderr)

        return _hook

    mod = types.ModuleType("antenv.axon_hooks")
    mod.get_axon_ntff_profile_hook = _make_hook
    mod.set_axon_ntff_profile_hook = lambda h: None
    sys.modules["antenv.axon_hooks"] = mod


def _trace_kernel(tc, xn, xt, wqT, wk, wvT, woh, ident, out):
    import concourse.mybir as mybir
    from concourse.bass import ts

    nc = tc.nc
    f32 = mybir.dt.float32
    bf16 = mybir.dt.bfloat16

    with contextlib.ExitStack() as top:
        gsb_pool = top.enter_context(tc.tile_pool(name="gsb", bufs=DT))
        n1_pool = top.enter_context(tc.tile_pool(name="n1", bufs=DT))
        n2_pool = top.enter_context(tc.tile_pool(name="n2", bufs=DT))
        k1_pool = top.enter_context(tc.tile_pool(name="k1", bufs=DT))
        m2_pool = top.enter_context(tc.tile_pool(name="m2", bufs=DT))
        id_pool = top.enter_context(tc.tile_pool(name="idp", bufs=1))
        wq_pool = top.enter_context(tc.tile_pool(name="wq", bufs=DT))
        wk_pool = top.enter_context(tc.tile_pool(name="wk", bufs=DT))
        wv_pool = top.enter_context(tc.tile_pool(name="wv", bufs=DT))
        wo_pool = top.enter_context(tc.tile_pool(name="wo", bufs=DT))
        xt_pool = top.enter_context(tc.tile_pool(name="xt", bufs=DT))
        ost_pool = top.enter_context(tc.tile_pool(name="ost", bufs=8))
        ps_pool = top.enter_context(tc.tile_pool(name="ps", bufs=8, space="PSUM"))

        gsb = [gsb_pool.tile([P, D], bf16, name=f"g{i}", tag="gsb") for i in range(DT)]
        n1 = [n1_pool.tile([P, FH], bf16, name=f"n1_{i}", tag="n1") for i in range(DT)]
        n2 = [n2_pool.tile([P, FH], bf16, name=f"n2_{i}", tag="n2") for i in range(DT)]
        k1 = [k1_pool.tile([P, FH], bf16, name=f"k1_{i}", tag="k1") for i in range(DT)]
        m2 = [m2_pool.tile([P, FH], bf16, name=f"m2_{i}", tag="m2") for i in range(DT)]
        idt = id_pool.tile([P, P], bf16, name="idt", tag="idt")

        # DMA queues: gpsimd carries wvT then xt group 0 (N1 is the tensor
        # engine's opening act); woh rides sync ahead of xn; xn splits across
        # sync/scalar so G is never DMA-starved; wk/wqT follow behind. xt
        # groups 1-3 recycle xn's SBUF space once G is done.
        nc.gpsimd.dma_start(out=idt[:], in_=ident)
        wvb = [wv_pool.tile([P, D], bf16, name=f"wv{i}", tag="wv") for i in range(DT)]
        wob = [wo_pool.tile([P, FH], bf16, name=f"wo{i}", tag="wo") for i in range(DT)]
        for i in range(DT):
            nc.gpsimd.dma_start(out=wvb[i][:], in_=wvT[ts(i, P), :])
            nc.sync.dma_start(out=wob[i][:], in_=woh[ts(i, P), :])
        xtg_all = [
            [xt_pool.tile([P, D], bf16, name=f"xt0_{i}", tag="xt") for i in range(DT)]
        ]
        for it in range(DT):
            nc.gpsimd.dma_start(out=xtg_all[0][it][:], in_=xt[ts(it, P), ts(0, D)])

        with contextlib.ExitStack() as setup:
            xn_pool = setup.enter_context(tc.tile_pool(name="xn", bufs=ST))
            xnb = [xn_pool.tile([P, D], bf16, name=f"xn{i}", tag="xn") for i in range(ST)]
            for i in range(ST):
                q = nc.sync if i % 2 == 0 else nc.scalar
                q.dma_start(out=xnb[i][:], in_=xn[ts(i, P), :])
            wkb = [wk_pool.tile([P, D], bf16, name=f"wk{i}", tag="wk") for i in range(DT)]
            wqb = [wq_pool.tile([P, D], bf16, name=f"wq{i}", tag="wq") for i in range(DT)]
            for i in range(DT):
                nc.sync.dma_start(out=wkb[i][:], in_=wk[ts(i, P), :])
                nc.scalar.dma_start(out=wqb[i][:], in_=wqT[ts(i, P), :])

            # --- N1 = wv @ wo[:, half] (tensor warms up while xn streams) ---
            pss = [ps_pool.tile([P, FREE], f32, name="psn1", tag="ps") for _ in range(DT)]
            for dt in range(DT):
                for eb in range(DT):
                    nc.tensor.matmul(
                        pss[eb][:],
                        wvb[dt][:, ts(eb, P)],
                        wob[dt][:],
                        start=(dt == 0),
                        stop=(dt == DT - 1),
                    )
            for eb in range(DT):
                nc.vector.tensor_copy(n1[eb][:], pss[eb][:])

            # --- G upper triangle: two streaming passes over the sequence ---
            for chunks in G_PASS:
                pss = {
                    (jt, c): ps_pool.tile([P, FREE], f32, name="psg", tag="ps")
                    for (jt, c, es, w) in chunks
                }
                for tt in range(ST):
                    for jt, c, es, w in chunks:
                        nc.tensor.matmul(
                            pss[jt, c][:, :w],
                            xnb[tt][:, ts(jt, P)],
                            xnb[tt][:, es : es + w],
                            start=(tt == 0),
                            stop=(tt == ST - 1),
                        )
                for jt, c, es, w in chunks:
                    nc.vector.tensor_copy(gsb[jt][:, es : es + w], pss[jt, c][:, :w])

            # --- mirror the lower-triangle blocks: G[jt,eb] = G[eb,jt].T ---
            for jt in range(1, DT):
                for eb in range(jt):
                    pst = ps_pool.tile([P, P], bf16, name="pst", tag="ps")
                    nc.tensor.transpose(pst[:], gsb[eb][:, ts(jt, P)], idt[:])
                    nc.vector.tensor_copy(gsb[jt][:, ts(eb, P)], pst[:])

        # xt groups 1-3 into the SBUF space xn vacated.
        xt2_pool = top.enter_context(tc.tile_pool(name="xt2", bufs=3 * DT))
        for g in range(1, 4):
            xtg = [
                xt2_pool.tile([P, D], bf16, name=f"xt{g}_{i}", tag="xt2")
                for i in range(DT)
            ]
            for it in range(DT):
                nc.gpsimd.dma_start(out=xtg[it][:], in_=xt[ts(it, P), ts(g, D)])
            xtg_all.append(xtg)

        # --- N2 = G @ N1 (lhsT=G works because G is symmetric) ---
        pss = [ps_pool.tile([P, FREE], f32, name="psn2", tag="ps") for _ in range(DT)]
        for et in range(DT):
            for db in range(DT):
                nc.tensor.matmul(
                    pss[db][:],
                    gsb[et][:, ts(db, P)],
                    n1[et][:],
                    start=(et == 0),
                    stop=(et == DT - 1),
                )
        for db in range(DT):
            nc.scalar.copy(n2[db][:], pss[db][:])

        # --- K1 = wk.T @ N2 ---
        pss = [ps_pool.tile([P, FREE], f32, name="psk1", tag="ps") for _ in range(DT)]
        for dt in range(DT):
            for cb in range(DT):
                nc.tensor.matmul(
                    pss[cb][:],
                    wkb[dt][:, ts(cb, P)],
                    n2[dt][:],
                    start=(dt == 0),
                    stop=(dt == DT - 1),
                )
        for cb in range(DT):
            nc.vector.tensor_copy(k1[cb][:], pss[cb][:])

        # --- M2 = wq @ K1 ---
        pss = [ps_pool.tile([P, FREE], f32, name="psm2", tag="ps") for _ in range(DT)]
        for ct in range(DT):
            for ib in range(DT):
                nc.tensor.matmul(
                    pss[ib][:],
                    wqb[ct][:, ts(ib, P)],
                    k1[ct][:],
                    start=(ct == 0),
                    stop=(ct == DT - 1),
                )
        for ib in range(DT):
            nc.scalar.copy(m2[ib][:], pss[ib][:])

        # --- out[:, half] = x @ M2, streaming xt column-groups of 1024 ---
        for g in range(4):
            xtg = xtg_all[g]
            pss = [ps_pool.tile([P, FREE], f32, name="pso", tag="ps") for _ in range(8)]
            for it in range(DT):
                for tb in range(8):
                    nc.tensor.matmul(
                        pss[tb][:],
                        xtg[it][:, ts(tb, P)],
                        m2[it][:],
                        start=(it == 0),
                        stop=(it == DT - 1),
                    )
            # Tail latency: alternate evac engines and store queues so the
            # last group's flush isn't serialized on one engine.
            for tb in range(8):
                ot = ost_pool.tile([P, FREE], f32, name="ot", tag="ost")
                if tb % 2 == 0:
                    nc.scalar.copy(ot[:], pss[tb][:])
                    nc.scalar.dma_start(out=out[ts(g * 8 + tb, P), :], in_=ot[:])
                else:
                    nc.vector.tensor_copy(ot[:], pss[tb][:])
                    nc.sync.dma_start(out=out[ts(g * 8 + tb, P), :], in_=ot[:])


def _build():
    _install_axon_ntff_shim()
    import concourse.mybir as mybir
    import concourse.tile as tile
    from concourse import bacc

    f32 = mybir.dt.float32
    bf16 = mybir.dt.bfloat16
    nc = bacc.Bacc("TRN2", target_bir_lowering=False, debug=False, num_devices=NCORES)
    xn = nc.dram_tensor("xn", [T, D], bf16, kind="ExternalInput").ap()
    xt = nc.dram_tensor("xt", [D, T], bf16, kind="ExternalInput").ap()
    wqT = nc.dram_tensor("wqT", [D, D], bf16, kind="ExternalInput").ap()
    wk = nc.dram_tensor("wk", [D, D], bf16, kind="ExternalInput").ap()
    wvT = nc.dram_tensor("wvT", [D, D], bf16, kind="ExternalInput").ap()
    woh = nc.dram_tensor("woh", [D, FH], bf16, kind="ExternalInput").ap()
    ident = nc.dram_tensor("ident", [P, P], bf16, kind="ExternalInput").ap()
    out = nc.dram_tensor("out", [T, FH], f32, kind="ExternalOutput").ap()

    with tile.TileContext(nc) as tc:
        _trace_kernel(tc, xn, xt, wqT, wk, wvT, woh, ident, out)
    nc.compile()
    return nc


def kernel(x, w_q, w_k, w_v, w_o):
    global LAST_RESULTS
    import ml_dtypes
    from concourse import bass_utils

    bf16 = ml_dtypes.bfloat16

    if "nc" not in _STATE:
        _STATE["nc"] = _build()
    nc = _STATE["nc"]

    x = np.ascontiguousarray(x, dtype=np.float32)
    wqT = np.ascontiguousarray(np.asarray(w_q, dtype=np.float32).T).astype(bf16)
    wkb = np.ascontiguousarray(np.asarray(w_k, dtype=np.float32)).astype(bf16)
    wvT = np.ascontiguousarray(np.asarray(w_v, dtype=np.float32).T).astype(bf16)
    wo = np.asarray(w_o, dtype=np.float32)
    wo_halves = [
        np.ascontiguousarray(wo[:, :FH]).astype(bf16),
        np.ascontiguousarray(wo[:, FH:]).astype(bf16),
    ]
    ident = np.eye(P, dtype=np.float32).astype(bf16)
    xn_b = [x[b].astype(bf16) for b in range(B)]
    xt_b = [np.ascontiguousarray(x[b].T).astype(bf16) for b in range(B)]

    in_maps = []
    for core in range(NCORES):
        b, fh = core // 2, core % 2
        in_maps.append(
            {
                "xn": xn_b[b],
                "xt": xt_b[b],
                "wqT": wqT,
                "wk": wkb,
                "wvT": wvT,
                "woh": wo_halves[fh],
                "ident": ident,
            }
        )

    LAST_RESULTS = bass_utils.run_bass_kernel_spmd(
        nc, in_maps, core_ids=list(range(NCORES))
    )
    out = np.empty((B, T, D), dtype=np.float32)
    for core in range(NCORES):
        b, fh = core // 2, core % 2
        out[b, :, fh * FH : (fh + 1) * FH] = LAST_RESULTS.results[core]["out"]
    return out


# revision 18
# speedup vs baseline: 4.0810x; 1.0024x over previous
"""Trainium2 Bass kernel: unnormalized single-head attention block.

Computes, for x [4, 4096, 1024] and w_q/w_k/w_v/w_o [1024, 1024] (all fp32):
    q = x @ w_q ; k = x @ w_k ; v = x @ w_v
    scores = q @ k.T            (no softmax)
    out = (scores @ v) @ w_o

There is no softmax, so matmul associativity applies:
    out_b = x_b @ (w_q @ w_k.T @ G_b @ w_v @ w_o),   G_b = x_b.T @ x_b
which drops the arithmetic from ~413 GFLOP (two [T,T] products) to ~90 GFLOP.
G is symmetric, so only its upper-triangle 128-blocks are computed directly;
the lower blocks are PE transposes of the upper ones (~3us vs ~48us of MMs).
The weight chain is right-associated against the core's 512-column slice of
w_o, so every factor is a [D,D] x [D,512] product (4 x 13.7us; no [D,D]x[D,D]
products at all):
    M2 = wq @ (wk.T @ (G @ (wv @ woh)))

Sharding: 8 NeuronCores = (4 batches) x (2 output-column halves). Each core
computes G_b over the full sequence, its M2 slice, and out[:, half] = x @ M2.
No collectives, no inter-core deps. (A pair-AllReduce version that halves the
G work was measured SLOWER: the CC op costs ~29us at ~40GB/s bus plus ~25us
of startup latency, which cannot hide behind the ~41us of independent work.)

Device math is bf16 (host-cast) with fp32 PSUM accumulation. Layout chain
(lhsT's partition dim is always the contraction dim):
    G[d,e]   = sum_t  xn[t,d]  xn[t,e]      lhsT=xn tile,   rhs=xn tile
    N1[e,f]  = sum_k  wvT[k,e] woh[k,f]     lhsT=wvT,       rhs=woh
    N2[d,f]  = sum_e  G[d,e]   N1[e,f]      lhsT=G (sym),   rhs=N1
    K1[c,f]  = sum_d  wk[d,c]  N2[d,f]      lhsT=wk,        rhs=N2
    M2[i,f]  = sum_c  wqT[c,i] K1[c,f]      lhsT=wqT,       rhs=K1
    out[t,f] = sum_i  xt[i,t]  M2[i,f]      lhsT=xt tile,   rhs=M2
"""

import contextlib
import ctypes
import os
import sys
import types

import numpy as np

B = 4
T = 4096
D = 1024
P = 128             # SBUF partitions
NCORES = 8
DT = D // P         # 8 tiles along any 1024 dim
ST = T // P         # 32 tiles along the sequence
FREE = 512          # PSUM bank width (fp32)
FH = D // 2         # 512 output columns per core

# Upper-triangle chunk table for symmetric G: (jt, psum chunk, e-start, width).
G_CHUNKS = []
for _jt in range(DT):
    for _c in range(2):
        _es = max(_c * FREE, _jt * P)
        _w = (_c + 1) * FREE - _es
        if _w > 0:
            G_CHUNKS.append((_jt, _c, _es, _w))
G_PASS = [[ch for ch in G_CHUNKS if ch[0] < 4], [ch for ch in G_CHUNKS if ch[0] >= 4]]

_STATE = {}
LAST_RESULTS = None


def _install_axon_ntff_shim():
    """bass_utils(trace=True) under axon imports antenv.axon_hooks, which the
    agent image lacks. Provide the documented ctypes equivalent so tracing
    works; degrades to hook=None when the .so has no profile symbols."""
    try:
        import antenv.axon_hooks  # noqa: F401
        return
    except ImportError:
        pass

    so_path = "/opt/axon/libaxon_pjrt.so"

    def _make_hook():
        try:
            lib = ctypes.CDLL(so_path)
        except OSError:
            return None
        if not hasattr(lib, "axon_start_nrt_profile"):
            return None
        lib.axon_start_nrt_profile.argtypes = [
            ctypes.POINTER(ctypes.c_int64),
            ctypes.c_size_t,
        ]
        lib.axon_start_nrt_profile.restype = ctypes.c_int64
        lib.axon_stop_nrt_profile.argtypes = [ctypes.c_char_p]
        lib.axon_stop_nrt_profile.restype = ctypes.c_int64

        @contextlib.contextmanager
        def _hook(output_dir, device_ids):
            import jax

            jax.devices()
            if device_ids:
                ids = (ctypes.c_int64 * len(device_ids))(*device_ids)
                rc = lib.axon_start_nrt_profile(ids, len(device_ids))
            else:
                rc = lib.axon_start_nrt_profile(None, 0)
            if rc != 0:
                raise RuntimeError(f"axon_start_nrt_profile rc={rc}")
            try:
                yield
            finally:
                n = lib.axon_stop_nrt_profile(str(output_dir).encode())
                print(f"profile: {n} file(s) written to {output_dir}", file=sys.stderr)

        return _hook

    mod = types.ModuleType("antenv.axon_hooks")
    mod.get_axon_ntff_profile_hook = _make_hook
    mod.set_axon_ntff_profile_hook = lambda h: None
    sys.modules["antenv.axon_hooks"] = mod


def _trace_kernel(tc, xn, xt, wqT, wk, wvT, woh, ident, out):
    import concourse.mybir as mybir
    from concourse.bass import ts

    nc = tc.nc
    f32 = mybir.dt.float32
    bf16 = mybir.dt.bfloat16

    with contextlib.ExitStack() as top:
        gsb_pool = top.enter_context(tc.tile_pool(name="gsb", bufs=DT))
        n1_pool = top.enter_context(tc.tile_pool(name="n1", bufs=DT))
        n2_pool = top.enter_context(tc.tile_pool(name="n2", bufs=DT))
        k1_pool = top.enter_context(tc.tile_pool(name="k1", bufs=DT))
        m2_pool = top.enter_context(tc.tile_pool(name="m2", bufs=DT))
        id_pool = top.enter_context(tc.tile_pool(name="idp", bufs=1))
        wq_pool = top.enter_context(tc.tile_pool(name="wq", bufs=DT))
        wk_pool = top.enter_context(tc.tile_pool(name="wk", bufs=DT))
        wv_pool = top.enter_context(tc.tile_pool(name="wv", bufs=DT))
        wo_pool = top.enter_context(tc.tile_pool(name="wo", bufs=DT))
        xt_pool = top.enter_context(tc.tile_pool(name="xt", bufs=DT))
        ost_pool = top.enter_context(tc.tile_pool(name="ost", bufs=8))
        ps_pool = top.enter_context(tc.tile_pool(name="ps", bufs=8, space="PSUM"))

        gsb = [gsb_pool.tile([P, D], bf16, name=f"g{i}", tag="gsb") for i in range(DT)]
        n1 = [n1_pool.tile([P, FH], bf16, name=f"n1_{i}", tag="n1") for i in range(DT)]
        n2 = [n2_pool.tile([P, FH], bf16, name=f"n2_{i}", tag="n2") for i in range(DT)]
        k1 = [k1_pool.tile([P, FH], bf16, name=f"k1_{i}", tag="k1") for i in range(DT)]
        m2 = [m2_pool.tile([P, FH], bf16, name=f"m2_{i}", tag="m2") for i in range(DT)]
        idt = id_pool.tile([P, P], bf16, name="idt", tag="idt")

        # DMA queues: gpsimd carries wvT then xt group 0 (N1 is the tensor
        # engine's opening act); woh rides sync ahead of xn; xn splits across
        # sync/scalar so G is never DMA-starved; wk/wqT follow behind. xt
        # groups 1-3 recycle xn's SBUF space once G is done.
        wvb = [wv_pool.tile([P, D], bf16, name=f"wv{i}", tag="wv") for i in range(DT)]
        wob = [wo_pool.tile([P, FH], bf16, name=f"wo{i}", tag="wo") for i in range(DT)]
        for i in range(DT):
            nc.gpsimd.dma_start(out=wvb[i][:], in_=wvT[ts(i, P), :])
            nc.sync.dma_start(out=wob[i][:], in_=woh[ts(i, P), :])
        nc.gpsimd.dma_start(out=idt[:], in_=ident)  # only needed at ~100us
        xtg_all = [
            [xt_pool.tile([P, D], bf16, name=f"xt0_{i}", tag="xt") for i in range(DT)]
        ]
        for it in range(DT):
            nc.gpsimd.dma_start(out=xtg_all[0][it][:], in_=xt[ts(it, P), ts(0, D)])

        with contextlib.ExitStack() as setup:
            xn_pool = setup.enter_context(tc.tile_pool(name="xn", bufs=ST))
            xnb = [xn_pool.tile([P, D], bf16, name=f"xn{i}", tag="xn") for i in range(ST)]
            for i in range(ST):
                q = nc.sync if i % 2 == 0 else nc.scalar
                q.dma_start(out=xnb[i][:], in_=xn[ts(i, P), :])
            wkb = [wk_pool.tile([P, D], bf16, name=f"wk{i}", tag="wk") for i in range(DT)]
            wqb = [wq_pool.tile([P, D], bf16, name=f"wq{i}", tag="wq") for i in range(DT)]
            for i in range(DT):
                nc.sync.dma_start(out=wkb[i][:], in_=wk[ts(i, P), :])
                nc.scalar.dma_start(out=wqb[i][:], in_=wqT[ts(i, P), :])

            # --- N1 = wv @ wo[:, half] (tensor warms up while xn streams) ---
            pss = [ps_pool.tile([P, FREE], f32, name="psn1", tag="ps") for _ in range(DT)]
            for dt in range(DT):
                for eb in range(DT):
                    nc.tensor.matmul(
                        pss[eb][:],
                        wvb[dt][:, ts(eb, P)],
                        wob[dt][:],
                        start=(dt == 0),
                        stop=(dt == DT - 1),
                    )
            for eb in range(DT):
                nc.vector.tensor_copy(n1[eb][:], pss[eb][:])

            # --- G upper triangle: two streaming passes over the sequence ---
            for chunks in G_PASS:
                pss = {
                    (jt, c): ps_pool.tile([P, FREE], f32, name="psg", tag="ps")
                    for (jt, c, es, w) in chunks
                }
                for tt in range(ST):
                    for jt, c, es, w in chunks:
                        nc.tensor.matmul(
                            pss[jt, c][:, :w],
                            xnb[tt][:, ts(jt, P)],
                            xnb[tt][:, es : es + w],
                            start=(tt == 0),
                            stop=(tt == ST - 1),
                        )
                for jt, c, es, w in chunks:
                    nc.vector.tensor_copy(gsb[jt][:, es : es + w], pss[jt, c][:, :w])

            # --- mirror the lower-triangle blocks: G[jt,eb] = G[eb,jt].T ---
            for jt in range(1, DT):
                for eb in range(jt):
                    pst = ps_pool.tile([P, P], bf16, name="pst", tag="ps")
                    nc.tensor.transpose(pst[:], gsb[eb][:, ts(jt, P)], idt[:])
                    nc.vector.tensor_copy(gsb[jt][:, ts(eb, P)], pst[:])

        # xt groups 1-3 into the SBUF space xn vacated.
        xt2_pool = top.enter_context(tc.tile_pool(name="xt2", bufs=3 * DT))
        for g in range(1, 4):
            xtg = [
                xt2_pool.tile([P, D], bf16, name=f"xt{g}_{i}", tag="xt2")
                for i in range(DT)
            ]
            for it in range(DT):
                nc.gpsimd.dma_start(out=xtg[it][:], in_=xt[ts(it, P), ts(g, D)])
            xtg_all.append(xtg)

        # --- N2 = G @ N1 (lhsT=G works because G is symmetric) ---
        pss = [ps_pool.tile([P, FREE], f32, name="psn2", tag="ps") for _ in range(DT)]
        for et in range(DT):
            for db in range(DT):
                nc.tensor.matmul(
                    pss[db][:],
                    gsb[et][:, ts(db, P)],
                    n1[et][:],
                    start=(et == 0),
                    stop=(et == DT - 1),
                )
        for db in range(DT):
            nc.scalar.copy(n2[db][:], pss[db][:])

        # --- K1 = wk.T @ N2 ---
        pss = [ps_pool.tile([P, FREE], f32, name="psk1", tag="ps") for _ in range(DT)]
        for dt in range(DT):
            for cb in range(DT):
                nc.tensor.matmul(
                    pss[cb][:],
                    wkb[dt][:, ts(cb, P)],
                    n2[dt][:],
                    start=(dt == 0),
                    stop=(dt == DT - 1),
                )
        for cb in range(DT):
            nc.vector.tensor_copy(k1[cb][:], pss[cb][:])

        # --- M2 = wq @ K1 ---
        pss = [ps_pool.tile([P, FREE], f32, name="psm2", tag="ps") for _ in range(DT)]
        for ct in range(DT):
            for ib in range(DT):
                nc.tensor.matmul(
                    pss[ib][:],
                    wqb[ct][:, ts(ib, P)],
                    k1[ct][:],
                    start=(ct == 0),
                    stop=(ct == DT - 1),
                )
        for ib in range(DT):
            nc.scalar.copy(m2[ib][:], pss[ib][:])

        # --- out[:, half] = x @ M2, streaming xt column-groups of 1024 ---
        for g in range(4):
            xtg = xtg_all[g]
            pss = [ps_pool.tile([P, FREE], f32, name="pso", tag="ps") for _ in range(8)]
            for it in range(DT):
                for tb in range(8):
                    nc.tensor.matmul(
                        pss[tb][:],
                        xtg[it][:, ts(tb, P)],
                        m2[it][:],
                        start=(it == 0),
                        stop=(it == DT - 1),
                    )
            # Tail latency: alternate evac engines and store queues so the
            # last group's flush isn't serialized on one engine.
            for tb in range(8):
                ot = ost_pool.tile([P, FREE], f32, name="ot", tag="ost")
                if tb % 2 == 0:
                    nc.scalar.copy(ot[:], pss[tb][:])
                    nc.scalar.dma_start(out=out[ts(g * 8 + tb, P), :], in_=ot[:])
                else:
                    nc.vector.tensor_copy(ot[:], pss[tb][:])
                    nc.sync.dma_start(out=out[ts(g * 8 + tb, P), :], in_=ot[:])


def _build():
    _install_axon_ntff_shim()
    import concourse.mybir as mybir
    import concourse.tile as tile
    from concourse import bacc

    f32 = mybir.dt.float32
    bf16 = mybir.dt.bfloat16
    nc = bacc.Bacc("TRN2", target_bir_lowering=False, debug=False, num_devices=NCORES)
    xn = nc.dram_tensor("xn", [T, D], bf16, kind="ExternalInput").ap()
    xt = nc.dram_tensor("xt", [D, T], bf16, kind="ExternalInput").ap()
    wqT = nc.dram_tensor("wqT", [D, D], bf16, kind="ExternalInput").ap()
    wk = nc.dram_tensor("wk", [D, D], bf16, kind="ExternalInput").ap()
    wvT = nc.dram_tensor("wvT", [D, D], bf16, kind="ExternalInput").ap()
    woh = nc.dram_tensor("woh", [D, FH], bf16, kind="ExternalInput").ap()
    ident = nc.dram_tensor("ident", [P, P], bf16, kind="ExternalInput").ap()
    out = nc.dram_tensor("out", [T, FH], f32, kind="ExternalOutput").ap()

    with tile.TileContext(nc) as tc:
        _trace_kernel(tc, xn, xt, wqT, wk, wvT, woh, ident, out)
    nc.compile()
    return nc


def kernel(x, w_q, w_k, w_v, w_o):
    global LAST_RESULTS
    import ml_dtypes
    from concourse import bass_utils

    bf16 = ml_dtypes.bfloat16

    if "nc" not in _STATE:
        _STATE["nc"] = _build()
    nc = _STATE["nc"]

    x = np.ascontiguousarray(x, dtype=np.float32)
    wqT = np.ascontiguousarray(np.asarray(w_q, dtype=np.float32).T).astype(bf16)
    wkb = np.ascontiguousarray(np.asarray(w_k, dtype=np.float32)).astype(bf16)
    wvT = np.ascontiguousarray(np.asarray(w_v, dtype=np.float32).T).astype(bf16)
    wo = np.asarray(w_o, dtype=np.float32)
    wo_halves = [
        np.ascontiguousarray(wo[:, :FH]).astype(bf16),
        np.ascontiguousarray(wo[:, FH:]).astype(bf16),
    ]
    ident = np.eye(P, dtype=np.float32).astype(bf16)
    xn_b = [x[b].astype(bf16) for b in range(B)]
    xt_b = [np.ascontiguousarray(x[b].T).astype(bf16) for b in range(B)]

    in_maps = []
    for core in range(NCORES):
        b, fh = core // 2, core % 2
        in_maps.append(
            {
                "xn": xn_b[b],
                "xt": xt_b[b],
                "wqT": wqT,
                "wk": wkb,
                "wvT": wvT,
                "woh": wo_halves[fh],
                "ident": ident,
            }
        )

    LAST_RESULTS = bass_utils.run_bass_kernel_spmd(
        nc, in_maps, core_ids=list(range(NCORES))
    )
    out = np.empty((B, T, D), dtype=np.float32)
    for core in range(NCORES):
        b, fh = core // 2, core % 2
        out[b, :, fh * FH : (fh + 1) * FH] = LAST_RESULTS.results[core]["out"]
    return out
